# revision 1
# baseline (speedup 1.0000x reference)
"""Trainium2 Bass kernel for nn_MultiHeadAttention_39135742001649.

Reference computation (B=2, S=2048, D=1024, H=16, WIN=512):
    q/k/v = x @ W.T + b (per-head dk=64)
    scores = q k^T / 8                               [B,H,S,S]
    probs1 = blockwise softmax: causal mask, softmax within each 512-wide
             column block (masked entries -> 0)
    probs2 = full-row softmax(probs1)  (no masking; exp(0)=1 entries!)
    out    = (probs2 @ v) @ Wo.T + bo

Decomposition used here (validated to 8e-7 rel err vs reference in fp32):
    e1   = exp(scores) * tril_mask        (only 10 of 16 causal blocks)
    d1   = colsum of e1 within block      -> probs1 = e1 / d1
    e2   = exp(probs1)                    (masked/uncomputed entries -> 1)
    out_row = (sum_causal e2 @ v + suffix_colsum_v) / (sum_causal e2 + 512*(3-bi))

Sharding: 8 cores = 2 batches x 4 head-groups (4 heads each). Each core
computes q^T/k^T/v for its heads, the attention, and a partial output
projection over its 256 d-rows; the host sums the 4 partials per batch.

All on-chip layouts are transposed ([c, q] / [d, s]) so matmul contraction
is on partitions. Matmuls run as float32r (full PE rate at free dim >= 256,
fp32 numerics).
"""

import numpy as np
from contextlib import ExitStack

import concourse.bass as bass
import concourse.mybir as mybir
import concourse.tile as tile
from concourse import bacc
from concourse.bass_utils import run_bass_kernel_spmd

F32 = mybir.dt.float32
F32R = mybir.dt.float32r
EXP = mybir.ActivationFunctionType.Exp
LN = mybir.ActivationFunctionType.Ln
ADD = mybir.AluOpType.add
MULT = mybir.AluOpType.mult

B, S, D, H, WIN = 2, 2048, 1024, 16, 512
DK = D // H          # 64
NB = S // WIN        # 4
NCORES = 8
HPC = 4              # heads per core
DCORE = HPC * DK     # 256
P = 128

TRACE = False        # set True from test.py to capture HW profile
USE_D2MM = False     # True: separate ones-matmul for d2; False: fold into PV via vE/vO ones cols
TRACE_CORES = None

_CACHE = {}


def _mm(nc, out, lhsT, rhs, start, stop):
    nc.tensor.matmul(out, lhsT, rhs, start=start, stop=stop)


def build_nc():
    nc = bacc.Bacc("TRN2", target_bir_lowering=False, debug=False)

    xT = nc.dram_tensor("xT", [D, S], F32R, kind="ExternalInput")        # x[b].T
    wqT = nc.dram_tensor("wqT", [D, DCORE], F32R, kind="ExternalInput")  # (Wq/8).T slice
    wkT = nc.dram_tensor("wkT", [D, DCORE], F32R, kind="ExternalInput")
    wvT = nc.dram_tensor("wvT", [D, DCORE], F32R, kind="ExternalInput")
    woT = nc.dram_tensor("woT", [DCORE, D], F32R, kind="ExternalInput")  # Wo.T row slice
    bq = nc.dram_tensor("bq", [DCORE], F32, kind="ExternalInput")       # /8
    bk = nc.dram_tensor("bk", [DCORE], F32, kind="ExternalInput")
    bvr = nc.dram_tensor("bvr", [P, DCORE], F32, kind="ExternalInput")  # bv replicated
    maskd = nc.dram_tensor("maskd", [NB, P, WIN], F32, kind="ExternalInput")
    onesd = nc.dram_tensor("onesd", [P, 2048], F32R, kind="ExternalInput")
    sfxd = nc.dram_tensor("sfxd", [P, 2, NB], F32, kind="ExternalInput")
    outT = nc.dram_tensor("outT", [D, S], F32, kind="ExternalOutput")   # partial out^T

    with tile.TileContext(nc) as tc, ExitStack() as ctx:
        const = ctx.enter_context(tc.tile_pool(name="const", bufs=1))
        wpool = ctx.enter_context(tc.tile_pool(name="wpool", bufs=1))
        persist = ctx.enter_context(tc.tile_pool(name="persist", bufs=1))

        ones128 = const.tile([P, P], F32R, name="ones128")
        nc.sync.dma_start(ones128[:], onesd[:, 0:P])
        mask_sb = const.tile([P, NB, WIN], F32, name="mask_sb")
        nc.sync.dma_start(mask_sb[:], maskd[:].rearrange("m p q -> p m q"))
        bq_sb = const.tile([P, 2], F32, name="bq_sb")
        nc.sync.dma_start(bq_sb[:], bq[:].rearrange("(c p) -> p c", p=P))
        bk_sb = const.tile([P, 2], F32, name="bk_sb")
        nc.sync.dma_start(bk_sb[:], bk[:].rearrange("(c p) -> p c", p=P))
        bvr_sb = const.tile([P, DCORE], F32, name="bvr_sb")
        nc.sync.dma_start(bvr_sb[:], bvr[:])

        wq_sb = wpool.tile([P, 8, DCORE], F32R, name="wq_sb")
        nc.sync.dma_start(wq_sb[:], wqT[:].rearrange("(o p) d -> p o d", p=P))
        wk_sb = wpool.tile([P, 8, DCORE], F32R, name="wk_sb")
        nc.sync.dma_start(wk_sb[:], wkT[:].rearrange("(o p) d -> p o d", p=P))
        wv_sb = wpool.tile([P, 8, DCORE], F32R, name="wv_sb")
        nc.sync.dma_start(wv_sb[:], wvT[:].rearrange("(o p) d -> p o d", p=P))
        wo_sb = wpool.tile([P, 2, D], F32R, name="wo_sb")
        nc.sync.dma_start(wo_sb[:], woT[:].rearrange("(o p) e -> p o e", p=P))

        qT_sb = persist.tile([P, 2, S], F32R, name="qT_sb")    # [d%128, d//128, s]
        kT_sb = persist.tile([P, 2, S], F32R, name="kT_sb")
        # Per head-pair padded V tiles for the PV matmul: even head's v in
        # cols 0:64 with ones in 64:128 (d2 lands in psum rows 64:128);
        # odd head's v in cols 64:128 with ones in 0:64 (d2 in rows 0:64).
        vE_sb = persist.tile([P, 16, 2, P], F32R, name="vE_sb")
        vO_sb = persist.tile([P, 16, 2, P], F32R, name="vO_sb")
        nc.sync.dma_start(vE_sb[:, :, :, DK:P],
                          onesd[:].rearrange("p (s c k) -> p s c k", s=16, c=2))
        nc.sync.dma_start(vO_sb[:, :, :, 0:DK],
                          onesd[:].rearrange("p (s c k) -> p s c k", s=16, c=2))
        attnT_sb = persist.tile([P, 2, S], F32R, name="attnT_sb")
        sfx_sb = const.tile([P, 2, NB], F32, name="sfx_sb")    # suffix sums (host)
        nc.sync.dma_start(sfx_sb[:], sfxd[:])

        # ---------------- Phase A: projections ----------------
        with (
            tc.tile_pool(name="xp", bufs=1) as xp,
            tc.tile_pool(name="psQK", bufs=3, space="PSUM") as psQK,
            tc.tile_pool(name="psV", bufs=3, space="PSUM") as psV,
        ):
            x_sb = xp.tile([P, 8, S], F32R, name="x_sb")
            xTr = xT[:].rearrange("(o p) s -> p o s", p=P)
            for o in range(8):
                nc.sync.dma_start(x_sb[:, o, :], xTr[:, o, :])

            # q^T and k^T: [DCORE, S] as [128, 2, S]
            for w_sb, b_sb, dst in ((wq_sb, bq_sb, qT_sb), (wk_sb, bk_sb, kT_sb)):
                for dc in range(2):
                    for st in range(NB):
                        ps = psQK.tile([P, WIN], F32, name="qk_ps")
                        for o in range(8):
                            _mm(nc, ps[:], w_sb[:, o, dc * P:(dc + 1) * P],
                                x_sb[:, o, st * WIN:(st + 1) * WIN],
                                start=(o == 0), stop=(o == 7))
                        nc.vector.tensor_scalar_add(
                            dst[:, dc, st * WIN:(st + 1) * WIN], ps[:],
                            b_sb[:, dc:dc + 1])

            # v: [S, DCORE]; evict per head into the padded vE/vO tiles
            for sc in range(16):
                ps = psV.tile([P, DCORE], F32, name="v_ps")
                for o in range(8):
                    _mm(nc, ps[:], x_sb[:, o, sc * P:(sc + 1) * P], wv_sb[:, o, :],
                        start=(o == 0), stop=(o == 7))
                for hc in range(2):
                    e0 = (2 * hc) * DK
                    o0 = (2 * hc + 1) * DK
                    nc.vector.tensor_tensor(vE_sb[:, sc, hc, 0:DK],
                                            ps[:, e0:e0 + DK],
                                            bvr_sb[:, e0:e0 + DK], ADD)
                    nc.vector.tensor_tensor(vO_sb[:, sc, hc, DK:P],
                                            ps[:, o0:o0 + DK],
                                            bvr_sb[:, o0:o0 + DK], ADD)

        # ---------------- Phase B: attention (3-stage skewed pipeline) ----
        # Per block job (h, bi, j): A: scores+exp1(+mask); B1: d1, 1/d1 via
        # ACT exp(-ln), s2 mult, exp2; B2: PV (+d2) matmuls, fixup on last j.
        # Stages of consecutive jobs are interleaved in issue order so no
        # engine queue head-of-line blocks on another engine's round trip.
        jobs = [(h, bi, j) for h in range(HPC) for bi in range(NB)
                for j in range(bi + 1)]
        with (
            tc.tile_pool(name="e1p", bufs=3) as e1p,
            tc.tile_pool(name="s2p", bufs=2) as s2p,
            tc.tile_pool(name="e2p", bufs=3) as e2p,
            tc.tile_pool(name="drp", bufs=2) as drp,
            tc.tile_pool(name="d2sp", bufs=2) as d2sp,
            tc.tile_pool(name="psSC", bufs=4, space="PSUM") as psSC,
            tc.tile_pool(name="psD1", bufs=2, space="PSUM") as psD1,
            tc.tile_pool(name="psPV", bufs=1, space="PSUM") as psPV,
        ):
            if USE_D2MM:
                psD2 = ctx.enter_context(tc.tile_pool(name="psD2", bufs=1,
                                                      space="PSUM"))
            state = {}

            def stage_a(job):
                h, bi, j = job
                hc, hb = h // 2, (h % 2) * DK
                e1 = e1p.tile([P, NB, WIN], F32R, name="e1")
                for m in range(NB):
                    sc_ps = psSC.tile([P, WIN], F32, name="sc_ps")
                    lhsT = kT_sb[hb:hb + DK, hc,
                                 j * WIN + m * P: j * WIN + (m + 1) * P]
                    rhs = qT_sb[hb:hb + DK, hc, bi * WIN:(bi + 1) * WIN]
                    _mm(nc, sc_ps[:], lhsT, rhs, start=True, stop=True)
                    nc.scalar.activation(e1[:, m, :], sc_ps[:], EXP)
                if j == bi:
                    for m in range(NB):
                        nc.vector.tensor_tensor(e1[:, m, :], e1[:, m, :],
                                                mask_sb[:, m, :], MULT)
                state[job] = e1

            def stage_b1(job):
                e1 = state.pop(job)
                d1_ps = psD1.tile([P, WIN], F32, name="d1_ps")
                for m in range(NB):
                    _mm(nc, d1_ps[:], ones128[:], e1[:, m, :],
                        start=(m == 0), stop=(m == 3))
                lnd1 = drp.tile([P, WIN], F32, name="lnd1")
                nc.scalar.activation(lnd1[:], d1_ps[:], LN)
                d1r = drp.tile([P, WIN], F32, name="d1r")
                nc.scalar.activation(d1r[:], lnd1[:], EXP, scale=-1.0)
                s2 = s2p.tile([P, NB, WIN], F32, name="s2")
                for m in range(NB):
                    nc.vector.tensor_tensor(s2[:, m, :], e1[:, m, :], d1r[:], MULT)
                e2 = e2p.tile([P, NB, WIN], F32R, name="e2")
                nc.scalar.activation(e2[:], s2[:], EXP)
                state[job] = e2

            def stage_b2(job):
                h, bi, j = job
                hc, hb = h // 2, (h % 2) * DK
                vh = vE_sb if h % 2 == 0 else vO_sb
                e2 = state.pop(job)
                if j == 0:
                    state[(h, bi, "pv")] = psPV.tile([P, WIN], F32, name="pv_ps")
                    if USE_D2MM:
                        state[(h, bi, "d2")] = psD2.tile([P, WIN], F32,
                                                         name="d2_ps")
                pv_ps = state[(h, bi, "pv")]
                first = (j == 0)
                last = (j == bi)
                for m in range(NB):
                    _mm(nc, pv_ps[:, :], vh[:, j * 4 + m, hc, :], e2[:, m, :],
                        start=(first and m == 0), stop=(last and m == 3))
                    if USE_D2MM:
                        _mm(nc, state[(h, bi, "d2")][:], ones128[:], e2[:, m, :],
                            start=(first and m == 0), stop=(last and m == 3))
                if not last:
                    return
                # fixup: attnT = (pv + sfx) / (d2 + 512*(3-bi))
                pv_ps = state.pop((h, bi, "pv"))
                d2s = d2sp.tile([P, WIN], F32, name="d2s")
                d2r = d2sp.tile([P, WIN], F32, name="d2r")
                cst = float(WIN * (NB - 1 - bi))
                if USE_D2MM:
                    d2_ps = state.pop((h, bi, "d2"))
                    nc.vector.tensor_scalar_add(d2s[hb:hb + DK, :],
                                                d2_ps[hb:hb + DK, :], cst)
                    nc.vector.reciprocal(d2r[hb:hb + DK, :], d2s[hb:hb + DK, :])
                    nc.vector.scalar_tensor_tensor(
                        attnT_sb[hb:hb + DK, hc, bi * WIN:(bi + 1) * WIN],
                        pv_ps[hb:hb + DK, :],
                        sfx_sb[hb:hb + DK, hc, bi:bi + 1],
                        d2r[hb:hb + DK, :], ADD, MULT)
                else:
                    opp = DK - hb  # d2 rows live at the opposite 64-row half
                    nc.vector.tensor_scalar_add(d2s[opp:opp + DK, :],
                                                pv_ps[opp:opp + DK, :], cst)
                    nc.vector.reciprocal(d2r[opp:opp + DK, :],
                                         d2s[opp:opp + DK, :])
                    nc.vector.scalar_tensor_tensor(
                        attnT_sb[hb:hb + DK, hc, bi * WIN:(bi + 1) * WIN],
                        pv_ps[hb:hb + DK, :],
                        sfx_sb[opp:opp + DK, hc, bi:bi + 1],
                        d2r[opp:opp + DK, :], ADD, MULT)

            n = len(jobs)
            for k in range(n + 2):
                if k < n:
                    stage_a(jobs[k])
                if 0 <= k - 1 < n:
                    stage_b1(jobs[k - 1])
                if 0 <= k - 2 < n:
                    stage_b2(jobs[k - 2])

        # ---------------- Phase C: output projection ----------------
        with (
            tc.tile_pool(name="otp", bufs=3) as otp,
            tc.tile_pool(name="psO", bufs=4, space="PSUM") as psO,
        ):
            for ec in range(8):
                for st in range(NB):
                    ps = psO.tile([P, WIN], F32, name="o_ps")
                    for dsub in range(2):
                        _mm(nc, ps[:], wo_sb[:, dsub, ec * P:(ec + 1) * P],
                            attnT_sb[:, dsub, st * WIN:(st + 1) * WIN],
                            start=(dsub == 0), stop=(dsub == 1))
                    ot = otp.tile([P, WIN], F32, name="ot")
                    nc.vector.tensor_copy(ot[:], ps[:])
                    nc.sync.dma_start(
                        outT[ec * P:(ec + 1) * P, st * WIN:(st + 1) * WIN], ot[:])

    nc.compile()
    return nc


def rnd12(a):
    """Round fp32 array to nearest float32r (12-bit mantissa)."""
    u = np.ascontiguousarray(a, np.float32).view(np.uint32)
    u = ((u.astype(np.uint64) + 0x400) & 0xFFFFF800).astype(np.uint32)
    return u.view(np.float32)


def make_in_maps(x, Wq_w, Wq_b, Wk_w, Wk_b, Wv_w, Wv_b, Wo_w, Wo_b):
    x = np.ascontiguousarray(np.asarray(x, np.float32))
    Wq8 = np.asarray(Wq_w, np.float32) / 8.0
    bq8 = np.asarray(Wq_b, np.float32) / 8.0
    wqT = rnd12(Wq8.T)
    wkT = rnd12(np.asarray(Wk_w, np.float32).T)
    wvT = rnd12(np.asarray(Wv_w, np.float32).T)
    woT = rnd12(np.asarray(Wo_w, np.float32).T)

    mask = np.zeros((NB, P, WIN), np.float32)
    for m in range(NB):
        c_idx = m * P + np.arange(P)[:, None]
        q_idx = np.arange(WIN)[None, :]
        mask[m] = (c_idx <= q_idx).astype(np.float32)

    xTb = [rnd12(x[b].T) for b in range(B)]

    in_maps = []
    for core in range(NCORES):
        b = core // 4
        h0 = (core % 4) * HPC
        dsl = slice(h0 * DK, (h0 + HPC) * DK)
        bv_core = np.asarray(Wv_b, np.float32)[dsl]
        # suffix colsum(v) table computed on host from the rounded operands:
        # colsum_j(v) = (sum_{s in block j} x[s,:]) @ WvT_core + 512*bv
        wvT_core = np.ascontiguousarray(wvT[:, dsl])
        rowsum = np.stack([xTb[b][:, j * WIN:(j + 1) * WIN].sum(axis=1)
                           for j in range(NB)])            # [NB, D]
        cs = rowsum @ wvT_core + WIN * bv_core[None, :]     # [NB, DCORE]
        sfx_full = np.zeros((NB, DCORE), np.float32)
        for bi in range(NB - 1):
            sfx_full[bi] = cs[bi + 1:].sum(axis=0)
        sfx = np.zeros((P, 2, NB), np.float32)
        for hc in range(2):
            for bi in range(NB):
                col = sfx_full[bi][hc * P:(hc + 1) * P]
                if USE_D2MM:
                    sfx[:, hc, bi] = col
                else:
                    # halves swapped: the fixup reads sfx at the d2 rows' base
                    sfx[0:DK, hc, bi] = col[DK:P]
                    sfx[DK:P, hc, bi] = col[0:DK]
        in_maps.append({
            "xT": xTb[b],
            "wqT": np.ascontiguousarray(wqT[:, dsl]),
            "wkT": np.ascontiguousarray(wkT[:, dsl]),
            "wvT": np.ascontiguousarray(wvT[:, dsl]),
            "woT": np.ascontiguousarray(woT[dsl, :]),
            "bq": np.ascontiguousarray(bq8[dsl]),
            "bk": np.ascontiguousarray(np.asarray(Wk_b, np.float32)[dsl]),
            "bvr": np.ascontiguousarray(np.broadcast_to(bv_core, (P, DCORE))),
            "maskd": mask,
            "onesd": np.ones((P, 2048), np.float32),
            "sfxd": sfx,
        })
    return in_maps


def kernel(**inputs):
    if "nc" not in _CACHE:
        _CACHE["nc"] = build_nc()
    nc = _CACHE["nc"]
    in_maps = make_in_maps(**inputs)
    kw = {}
    if TRACE:
        kw["trace"] = True
        if TRACE_CORES is not None:
            kw["trace_cores"] = TRACE_CORES
    res = run_bass_kernel_spmd(nc, in_maps, list(range(NCORES)), **kw)
    _CACHE["last_result"] = res

    bo = np.asarray(inputs["Wo_b"], np.float32)
    out = np.zeros((B, S, D), np.float32)
    for b in range(B):
        acc = np.zeros((D, S), np.float32)
        for core in range(b * 4, b * 4 + 4):
            acc += res.results[core]["outT"]
        out[b] = acc.T + bo
    return out



# revision 8
# speedup vs baseline: 1.5300x; 1.5300x over previous
"""Trainium2 Bass kernel for nn_MultiHeadAttention_39135742001649.

Reference computation (B=2, S=2048, D=1024, H=16, WIN=512):
    q/k/v = x @ W.T + b (per-head dk=64)
    scores = q k^T / 8                               [B,H,S,S]
    probs1 = blockwise softmax: causal mask, softmax within each 512-wide
             column block (masked entries -> 0)
    probs2 = full-row softmax(probs1)  (no masking; exp(0)=1 entries!)
    out    = (probs2 @ v) @ Wo.T + bo

Decomposition (validated vs reference):
    e1   = exp(scores) * tril_mask        (only 10 of 16 causal blocks)
    d1   = colsum of e1 within block      -> probs1 = e1 / d1
    e2   = exp(probs1)                    (masked/uncomputed entries -> 1)
    out_row = (sum_causal e2 @ v + suffix_colsum_v) / (sum_causal e2 + 512*(3-bi))

Sharding: 8 cores = 2 batches x 4 head-groups (4 heads each). Each core
computes q^T/k^T/v for its heads, the attention, and a partial output
projection over its 256 d-rows; the host sums the 4 partials per batch.

All on-chip layouts are transposed ([c, q] / [d, s]) so matmul contraction
is on partitions. Data path is bf16 (PSUM accumulation fp32): bf16 matmuls
run 1 cycle/row at any free size, DVE element-wise ops hit the 2x/4x packed
modes, and HBM traffic halves. 1/d on DVE via reciprocal_approx_fast
(no EXP<->LN activation-table swaps on the scalar engine).
"""

import numpy as np
from contextlib import ExitStack

import concourse.bass as bass
import concourse.mybir as mybir
import concourse.tile as tile
from concourse import bacc
from concourse.bass_utils import run_bass_kernel_spmd

F32 = mybir.dt.float32
BF16 = mybir.dt.bfloat16
EXP = mybir.ActivationFunctionType.Exp
ADD = mybir.AluOpType.add
MULT = mybir.AluOpType.mult
BYPASS = mybir.AluOpType.bypass

B, S, D, H, WIN = 2, 2048, 1024, 16, 512
DK = D // H          # 64
NB = S // WIN        # 4
NCORES = 8
HPC = 4              # heads per core
DCORE = HPC * DK     # 256
P = 128
NC_CHUNK = WIN // P  # 4 chunks of 128 keys per block

TRACE = False        # set True from test.py to capture HW profile
TRACE_CORES = None

_CACHE = {}


def build_nc():
    nc = bacc.Bacc("TRN2", target_bir_lowering=False, debug=False)

    xT = nc.dram_tensor("xT", [D, S], BF16, kind="ExternalInput")        # x[b].T
    wqT = nc.dram_tensor("wqT", [D, DCORE], BF16, kind="ExternalInput")  # (Wq/8).T slice
    wkT = nc.dram_tensor("wkT", [D, DCORE], BF16, kind="ExternalInput")
    wvT = nc.dram_tensor("wvT", [D, DCORE], BF16, kind="ExternalInput")
    woT = nc.dram_tensor("woT", [DCORE, D], BF16, kind="ExternalInput")  # Wo.T row slice
    bq = nc.dram_tensor("bq", [DCORE], F32, kind="ExternalInput")        # /8
    bk = nc.dram_tensor("bk", [DCORE], F32, kind="ExternalInput")
    bvr = nc.dram_tensor("bvr", [P, DCORE], F32, kind="ExternalInput")   # bv replicated
    maskd = nc.dram_tensor("maskd", [P, P], BF16, kind="ExternalInput")  # tril triangle
    sfxd = nc.dram_tensor("sfxd", [DK, 2, 2, NB], F32, kind="ExternalInput")
    outT = nc.dram_tensor("outT", [D, S], BF16, kind="ExternalOutput")   # partial out^T

    with tile.TileContext(nc) as tc, ExitStack() as ctx:
        const = ctx.enter_context(tc.tile_pool(name="const", bufs=1))
        wpool = ctx.enter_context(tc.tile_pool(name="wpool", bufs=1))
        persist = ctx.enter_context(tc.tile_pool(name="persist", bufs=1))

        mask_sb = const.tile([P, P], BF16, name="mask_sb")
        nc.sync.dma_start(mask_sb[:], maskd[:])
        bq_sb = const.tile([P, 2], F32, name="bq_sb")
        nc.sync.dma_start(bq_sb[:], bq[:].rearrange("(c p) -> p c", p=P))
        bk_sb = const.tile([P, 2], F32, name="bk_sb")
        nc.sync.dma_start(bk_sb[:], bk[:].rearrange("(c p) -> p c", p=P))
        bvr_sb = const.tile([P, DCORE], F32, name="bvr_sb")
        nc.sync.dma_start(bvr_sb[:], bvr[:])
        sfx_sb = const.tile([DK, 2, 2, NB], F32, name="sfx_sb")  # suffix sums
        nc.sync.dma_start(sfx_sb[:], sfxd[:])

        ones128 = const.tile([P, P], BF16, name="ones128")
        nc.gpsimd.memset(ones128[:], 1.0)

        wq_sb = wpool.tile([P, 8, DCORE], BF16, name="wq_sb")
        nc.sync.dma_start(wq_sb[:], wqT[:].rearrange("(o p) d -> p o d", p=P))
        wk_sb = wpool.tile([P, 8, DCORE], BF16, name="wk_sb")
        nc.sync.dma_start(wk_sb[:], wkT[:].rearrange("(o p) d -> p o d", p=P))
        wv_sb = wpool.tile([P, 8, DCORE], BF16, name="wv_sb")
        nc.sync.dma_start(wv_sb[:], wvT[:].rearrange("(o p) d -> p o d", p=P))
        wo_sb = wpool.tile([P, 2, D], BF16, name="wo_sb")
        nc.sync.dma_start(wo_sb[:], woT[:].rearrange("(o p) e -> p o e", p=P))

        qT_sb = persist.tile([P, 2, S], BF16, name="qT_sb")    # [d%128, d//128, s]
        kT_sb = persist.tile([P, 2, S], BF16, name="kT_sb")
        # Per head-pair padded V tiles for the PV matmul: even head's v in
        # cols 0:64 with ones in 64:128 (d2 lands in psum rows 64:128);
        # odd head's v in cols 64:128 with ones in 0:64 (d2 in rows 0:64).
        vE_sb = persist.tile([P, 16, 2, P], BF16, name="vE_sb")
        vO_sb = persist.tile([P, 16, 2, P], BF16, name="vO_sb")
        nc.gpsimd.memset(vE_sb[:, :, :, DK:P], 1.0)
        nc.gpsimd.memset(vO_sb[:, :, :, 0:DK], 1.0)
        attnT_sb = persist.tile([P, 2, S], BF16, name="attnT_sb")

        # ---------------- Phase A: projections ----------------
        # Issue order: round r handles q's block (3-r), k's block r, and v's
        # s-chunks [4r, 4r+4) so phase B (which starts at bi=3) unblocks
        # after round 0.
        with (
            tc.tile_pool(name="xp", bufs=1) as xp,
            tc.tile_pool(name="psQK", bufs=3, space="PSUM") as psQK,
            tc.tile_pool(name="psV", bufs=3, space="PSUM") as psV,
        ):
            x_sb = xp.tile([P, 8, S], BF16, name="x_sb")
            xTr = xT[:].rearrange("(o p) s -> p o s", p=P)
            for st in (3, 0, 1, 2):
                for o in range(8):
                    nc.sync.dma_start(x_sb[:, o, st * WIN:(st + 1) * WIN],
                                      xTr[:, o, st * WIN:(st + 1) * WIN])

            def qk_proj(w_sb, b_sb, dst, st):
                for dc in range(2):
                    ps = psQK.tile([P, WIN], F32, name="qk_ps")
                    for o in range(8):
                        nc.tensor.matmul(ps[:], w_sb[:, o, dc * P:(dc + 1) * P],
                                         x_sb[:, o, st * WIN:(st + 1) * WIN],
                                         start=(o == 0), stop=(o == 7))
                    nc.vector.tensor_scalar_add(
                        dst[:, dc, st * WIN:(st + 1) * WIN], ps[:],
                        b_sb[:, dc:dc + 1])

            def v_proj(sc):
                ps = psV.tile([P, DCORE], F32, name="v_ps")
                for o in range(8):
                    nc.tensor.matmul(ps[:], x_sb[:, o, sc * P:(sc + 1) * P],
                                     wv_sb[:, o, :],
                                     start=(o == 0), stop=(o == 7))
                for hc in range(2):
                    e0 = (2 * hc) * DK
                    o0 = (2 * hc + 1) * DK
                    nc.vector.tensor_tensor(vE_sb[:, sc, hc, 0:DK],
                                            ps[:, e0:e0 + DK],
                                            bvr_sb[:, e0:e0 + DK], ADD)
                    nc.vector.tensor_tensor(vO_sb[:, sc, hc, DK:P],
                                            ps[:, o0:o0 + DK],
                                            bvr_sb[:, o0:o0 + DK], ADD)

            for r in range(NB):
                qk_proj(wq_sb, bq_sb, qT_sb, 3 - r)
                qk_proj(wk_sb, bk_sb, kT_sb, r)
                for sc in range(4 * r, 4 * r + 4):
                    v_proj(sc)

        # ---------------- Phase B: attention (3-stage skewed pipeline) ----
        # Per block job (h, bi, j): A: scores+exp1(+mask); B1: d1 matmuls,
        # 1/d1 via DVE approx reciprocal, bf16 convert on gpsimd, s2 (4x stt),
        # exp2; B2: PV (+d2 via ones cols) matmuls, fixup on last j.
        # Jobs run bi-descending so the output projection for block st can be
        # interleaved as soon as all heads' (bi=st) groups complete.
        jobs = [(h, bi, j) for bi in (3, 2, 1, 0) for h in range(HPC)
                for j in range(bi + 1)]
        with (
            tc.tile_pool(name="e1p", bufs=3) as e1p,
            tc.tile_pool(name="s2p", bufs=2) as s2p,
            tc.tile_pool(name="e2p", bufs=3) as e2p,
            tc.tile_pool(name="drp", bufs=2) as drp,
            tc.tile_pool(name="d2sp", bufs=2) as d2sp,
            tc.tile_pool(name="otp", bufs=3) as otp,
            tc.tile_pool(name="psSC", bufs=2, space="PSUM") as psSC,
            tc.tile_pool(name="psD1", bufs=2, space="PSUM") as psD1,
            tc.tile_pool(name="psPV", bufs=2, space="PSUM") as psPV,
        ):
            state = {}

            def stage_a(job):
                h, bi, j = job
                hc, hb = h // 2, (h % 2) * DK
                diag = (j == bi)
                e1 = e1p.tile([P, NB, WIN], BF16, name="e1")
                for pair in range(2):
                    sc_ps = psSC.tile([P, 2, WIN], F32, name="sc_ps")
                    for mloc in range(2):
                        m = 2 * pair + mloc
                        lo = m * P if diag else 0
                        lhsT = kT_sb[hb:hb + DK, hc,
                                     j * WIN + m * P: j * WIN + (m + 1) * P]
                        rhs = qT_sb[hb:hb + DK, hc,
                                    bi * WIN + lo:(bi + 1) * WIN]
                        nc.tensor.matmul(sc_ps[:, mloc, lo:], lhsT, rhs,
                                         start=True, stop=True)
                    if diag:
                        for mloc in range(2):
                            m = 2 * pair + mloc
                            lo = m * P
                            nc.scalar.activation(e1[:, m, lo:],
                                                 sc_ps[:, mloc, lo:], EXP)
                    else:
                        nc.scalar.activation(e1[:, 2 * pair:2 * pair + 2, :],
                                             sc_ps[:], EXP)
                if diag:
                    for m in range(NB):
                        lo = m * P
                        nc.vector.tensor_tensor(e1[:, m, lo:lo + P],
                                                e1[:, m, lo:lo + P],
                                                mask_sb[:], MULT)
                state[job] = e1

            def stage_b1(job):
                h, bi, j = job
                diag = (j == bi)
                e1 = state.pop(job)
                d1_ps = psD1.tile([P, WIN], F32, name="d1_ps")
                for m in range(NB):
                    lo = m * P if diag else 0
                    nc.tensor.matmul(d1_ps[:, lo:], ones128[:], e1[:, m, lo:],
                                     start=(m == 0), stop=(m == 3))
                d1r = drp.tile([P, WIN], F32, name="d1r")
                nc.vector.reciprocal_approx_fast(d1r[:], d1_ps[:])
                d1rb = drp.tile([P, WIN], BF16, name="d1rb")
                nc.gpsimd.tensor_copy(d1rb[:], d1r[:])
                s2 = s2p.tile([P, NB, WIN], BF16, name="s2")
                e2 = e2p.tile([P, NB, WIN], BF16, name="e2")
                for m in range(NB):
                    lo = m * P if diag else 0
                    nc.vector.scalar_tensor_tensor(
                        s2[:, m, lo:], e1[:, m, lo:], 0.0, d1rb[:, lo:],
                        BYPASS, MULT)
                if diag:
                    for m in range(1, NB):
                        nc.gpsimd.memset(e2[:, m, 0:m * P], 1.0)
                    for m in range(NB):
                        lo = m * P
                        nc.scalar.activation(e2[:, m, lo:], s2[:, m, lo:], EXP)
                else:
                    nc.scalar.activation(e2[:], s2[:], EXP)
                state[job] = e2

            def stage_b2(job):
                h, bi, j = job
                hc, hb = h // 2, (h % 2) * DK
                vh = vE_sb if h % 2 == 0 else vO_sb
                e2 = state.pop(job)
                if j == 0:
                    state[(h, bi, "pv")] = psPV.tile([P, WIN], F32, name="pv_ps")
                pv_ps = state[(h, bi, "pv")]
                first = (j == 0)
                last = (j == bi)
                for m in range(NB):
                    nc.tensor.matmul(pv_ps[:, :], vh[:, j * 4 + m, hc, :],
                                     e2[:, m, :],
                                     start=(first and m == 0),
                                     stop=(last and m == 3))
                if not last:
                    return
                # fixup: attnT = (pv + sfx) / (d2 + 512*(3-bi))
                pv_ps = state.pop((h, bi, "pv"))
                d2s = d2sp.tile([P, WIN], F32, name="d2s")
                d2r = d2sp.tile([P, WIN], F32, name="d2r")
                cst = float(WIN * (NB - 1 - bi))
                opp = DK - hb  # d2 rows live at the opposite 64-row half
                nc.vector.tensor_scalar_add(d2s[0:DK, :],
                                            pv_ps[opp:opp + DK, :], cst)
                nc.vector.reciprocal_approx_fast(d2r[0:DK, :], d2s[0:DK, :])
                nc.vector.scalar_tensor_tensor(
                    attnT_sb[hb:hb + DK, hc, bi * WIN:(bi + 1) * WIN],
                    pv_ps[hb:hb + DK, :],
                    sfx_sb[0:DK, hb // DK, hc, bi:bi + 1],
                    d2r[0:DK, :], ADD, MULT)

            def out_proj(st):
                # output projection for query block st; needs attnT[:, :, st]
                for ec in range(8):
                    ps = psD1.tile([P, WIN], F32, name="d1_ps")
                    for dsub in range(2):
                        nc.tensor.matmul(
                            ps[:], wo_sb[:, dsub, ec * P:(ec + 1) * P],
                            attnT_sb[:, dsub, st * WIN:(st + 1) * WIN],
                            start=(dsub == 0), stop=(dsub == 1))
                    ot = otp.tile([P, WIN], BF16, name="ot")
                    nc.vector.tensor_copy(ot[:], ps[:])
                    nc.sync.dma_start(
                        outT[ec * P:(ec + 1) * P, st * WIN:(st + 1) * WIN],
                        ot[:])

            # bi-group boundaries in the job list (bi descending: 16,12,8,4)
            n = len(jobs)
            done_after = {}   # k index after which all of bi's fixups issued
            cum = 0
            for bi, cnt in ((3, 16), (2, 12), (1, 8), (0, 4)):
                cum += cnt
                done_after[cum] = bi
            for k in range(n + 2):
                if k < n:
                    stage_a(jobs[k])
                if 0 <= k - 1 < n:
                    stage_b1(jobs[k - 1])
                if 0 <= k - 2 < n:
                    stage_b2(jobs[k - 2])
                # out_proj(st) once all (h, st) fixups are issued (k-2 offset)
                if (k - 2) in done_after:
                    out_proj(done_after[k - 2])
            out_proj(0)

    nc.compile()
    return nc


def make_in_maps(x, Wq_w, Wq_b, Wk_w, Wk_b, Wv_w, Wv_b, Wo_w, Wo_b):
    from ml_dtypes import bfloat16

    def bfc(a):
        return np.ascontiguousarray(np.asarray(a, np.float32).astype(bfloat16))

    x = np.asarray(x, np.float32)
    Wq8 = np.asarray(Wq_w, np.float32) / 8.0
    bq8 = np.asarray(Wq_b, np.float32) / 8.0
    wqT = bfc(Wq8.T)
    wkT = bfc(np.asarray(Wk_w, np.float32).T)
    wvT = bfc(np.asarray(Wv_w, np.float32).T)
    woT = bfc(np.asarray(Wo_w, np.float32).T)

    # lower-triangle [128,128] mask for the in-chunk diagonal (k<=q)
    tri = (np.arange(P)[:, None] <= np.arange(P)[None, :]).astype(np.float32)
    maskb = bfc(tri)

    xTb = [bfc(x[b].T) for b in range(B)]

    in_maps = []
    for core in range(NCORES):
        b = core // 4
        h0 = (core % 4) * HPC
        dsl = slice(h0 * DK, (h0 + HPC) * DK)
        bv_core = np.asarray(Wv_b, np.float32)[dsl]
        # suffix colsum(v) table computed on host from the rounded operands:
        # colsum_j(v) = (sum_{s in block j} x[s,:]) @ WvT_core + 512*bv
        wvT_core = np.ascontiguousarray(wvT[:, dsl]).astype(np.float32)
        xb32 = xTb[b].astype(np.float32)
        rowsum = np.stack([xb32[:, j * WIN:(j + 1) * WIN].sum(axis=1)
                           for j in range(NB)])            # [NB, D]
        cs = rowsum @ wvT_core + WIN * bv_core[None, :]     # [NB, DCORE]
        sfx_full = np.zeros((NB, DCORE), np.float32)
        for bi in range(NB - 1):
            sfx_full[bi] = cs[bi + 1:].sum(axis=0)
        sfx = np.zeros((DK, 2, 2, NB), np.float32)
        for hc in range(2):
            for half in range(2):
                for bi in range(NB):
                    sfx[:, half, hc, bi] = sfx_full[bi][
                        hc * P + half * DK: hc * P + half * DK + DK]
        in_maps.append({
            "xT": xTb[b],
            "wqT": np.ascontiguousarray(wqT[:, dsl]),
            "wkT": np.ascontiguousarray(wkT[:, dsl]),
            "wvT": np.ascontiguousarray(wvT[:, dsl]),
            "woT": np.ascontiguousarray(woT[dsl, :]),
            "bq": np.ascontiguousarray(bq8[dsl]),
            "bk": np.ascontiguousarray(np.asarray(Wk_b, np.float32)[dsl]),
            "bvr": np.ascontiguousarray(np.broadcast_to(bv_core, (P, DCORE))),
            "maskd": maskb,
            "sfxd": sfx,
        })
    return in_maps


def kernel(**inputs):
    if "nc" not in _CACHE:
        _CACHE["nc"] = build_nc()
    nc = _CACHE["nc"]
    in_maps = make_in_maps(**inputs)
    kw = {}
    if TRACE:
        kw["trace"] = True
        if TRACE_CORES is not None:
            kw["trace_cores"] = TRACE_CORES
    res = run_bass_kernel_spmd(nc, in_maps, list(range(NCORES)), **kw)
    _CACHE["last_result"] = res

    bo = np.asarray(inputs["Wo_b"], np.float32)
    out = np.zeros((B, S, D), np.float32)
    for b in range(B):
        acc = np.zeros((D, S), np.float32)
        for core in range(b * 4, b * 4 + 4):
            acc += np.asarray(res.results[core]["outT"], np.float32)
        out[b] = acc.T + bo
    return out


# revision 9
# speedup vs baseline: 1.7385x; 1.1363x over previous
"""Trainium2 Bass kernel for nn_MultiHeadAttention_39135742001649.

Reference computation (B=2, S=2048, D=1024, H=16, WIN=512):
    q/k/v = x @ W.T + b (per-head dk=64)
    scores = q k^T / 8                               [B,H,S,S]
    probs1 = blockwise softmax: causal mask, softmax within each 512-wide
             column block (masked entries -> 0)
    probs2 = full-row softmax(probs1)  (no masking; exp(0)=1 entries!)
    out    = (probs2 @ v) @ Wo.T + bo

Decomposition (validated vs reference):
    e1   = exp(scores) * tril_mask        (only 10 of 16 causal blocks)
    d1   = colsum of e1 within block      -> probs1 = e1 / d1
    e2   = exp(probs1)                    (masked/uncomputed entries -> 1)
    out_row = (sum_causal e2 @ v + suffix_colsum_v) / (sum_causal e2 + 512*(3-bi))

Sharding: 8 cores = 2 batches x 4 head-groups (4 heads each). Each core
computes q^T/k^T/v for its heads, the attention, and a partial output
projection over its 256 d-rows; the host sums the 4 partials per batch.

Structure: work proceeds in 4 "rounds", one per query block bi (ascending).
Round r projects q(block r), k(block r), v(chunks 4r..4r+4), then runs all
attention jobs (h, bi=r, j<=r) through a 4-stage skewed pipeline
(scores+exp1 | d1+recip | s2+exp2 | PV+fixup), then the output projection
for block r.  This keeps the in-order PE queue fed from ~20us onward.

Data path is bf16 (PSUM accumulation fp32). 1/d via DVE
reciprocal_approx_fast (no activation-table swaps).
"""

import numpy as np
from contextlib import ExitStack

import concourse.bass as bass
import concourse.mybir as mybir
import concourse.tile as tile
from concourse import bacc
from concourse.bass_utils import run_bass_kernel_spmd

F32 = mybir.dt.float32
BF16 = mybir.dt.bfloat16
EXP = mybir.ActivationFunctionType.Exp
COPY = mybir.ActivationFunctionType.Copy
IDENT = mybir.ActivationFunctionType.Identity
ADD = mybir.AluOpType.add
MULT = mybir.AluOpType.mult
BYPASS = mybir.AluOpType.bypass

B, S, D, H, WIN = 2, 2048, 1024, 16, 512
DK = D // H          # 64
NB = S // WIN        # 4
NCORES = 8
HPC = 4              # heads per core
DCORE = HPC * DK     # 256
P = 128

TRACE = False        # set True from test.py to capture HW profile
TRACE_CORES = None

_CACHE = {}


def build_nc():
    nc = bacc.Bacc("TRN2", target_bir_lowering=False, debug=False)

    xT = nc.dram_tensor("xT", [D, S], BF16, kind="ExternalInput")        # x[b].T
    wqT = nc.dram_tensor("wqT", [D, DCORE], BF16, kind="ExternalInput")  # (Wq/8).T slice
    wkT = nc.dram_tensor("wkT", [D, DCORE], BF16, kind="ExternalInput")
    wvT = nc.dram_tensor("wvT", [D, DCORE], BF16, kind="ExternalInput")
    woT = nc.dram_tensor("woT", [DCORE, D], BF16, kind="ExternalInput")  # Wo.T row slice
    bq = nc.dram_tensor("bq", [DCORE], F32, kind="ExternalInput")        # /8
    bk = nc.dram_tensor("bk", [DCORE], F32, kind="ExternalInput")
    bvr = nc.dram_tensor("bvr", [P, DCORE], F32, kind="ExternalInput")   # bv replicated
    maskd = nc.dram_tensor("maskd", [P, P], BF16, kind="ExternalInput")  # tril triangle
    sfxd = nc.dram_tensor("sfxd", [DK, 2, 2, NB], F32, kind="ExternalInput")
    outT = nc.dram_tensor("outT", [D, S], BF16, kind="ExternalOutput")   # partial out^T

    with tile.TileContext(nc) as tc, ExitStack() as ctx:
        const = ctx.enter_context(tc.tile_pool(name="const", bufs=1))
        wpool = ctx.enter_context(tc.tile_pool(name="wpool", bufs=1))
        persist = ctx.enter_context(tc.tile_pool(name="persist", bufs=1))

        mask_sb = const.tile([P, P], BF16, name="mask_sb")
        nc.sync.dma_start(mask_sb[:], maskd[:])
        bq_sb = const.tile([P, 2], F32, name="bq_sb")
        nc.sync.dma_start(bq_sb[:], bq[:].rearrange("(c p) -> p c", p=P))
        bk_sb = const.tile([P, 2], F32, name="bk_sb")
        nc.sync.dma_start(bk_sb[:], bk[:].rearrange("(c p) -> p c", p=P))
        bvr_sb = const.tile([P, DCORE], F32, name="bvr_sb")
        nc.sync.dma_start(bvr_sb[:], bvr[:])
        sfx_sb = const.tile([DK, 2, 2, NB], F32, name="sfx_sb")  # suffix sums
        nc.sync.dma_start(sfx_sb[:], sfxd[:])

        ones128 = const.tile([P, P], BF16, name="ones128")
        nc.gpsimd.memset(ones128[:], 1.0)

        wq_sb = wpool.tile([P, 8, DCORE], BF16, name="wq_sb")
        nc.sync.dma_start(wq_sb[:], wqT[:].rearrange("(o p) d -> p o d", p=P))
        wk_sb = wpool.tile([P, 8, DCORE], BF16, name="wk_sb")
        nc.sync.dma_start(wk_sb[:], wkT[:].rearrange("(o p) d -> p o d", p=P))
        wv_sb = wpool.tile([P, 8, DCORE], BF16, name="wv_sb")
        nc.sync.dma_start(wv_sb[:], wvT[:].rearrange("(o p) d -> p o d", p=P))
        wo_sb = wpool.tile([P, 2, D], BF16, name="wo_sb")
        nc.sync.dma_start(wo_sb[:], woT[:].rearrange("(o p) e -> p o e", p=P))

        qT_sb = persist.tile([P, 2, S], BF16, name="qT_sb")    # [d%128, d//128, s]
        kT_sb = persist.tile([P, 2, S], BF16, name="kT_sb")
        # Per head-pair padded V tiles for the PV matmul: even head's v in
        # cols 0:64 with ones in 64:128 (d2 lands in psum rows 64:128);
        # odd head's v in cols 64:128 with ones in 0:64 (d2 in rows 0:64).
        vE_sb = persist.tile([P, 16, 2, P], BF16, name="vE_sb")
        vO_sb = persist.tile([P, 16, 2, P], BF16, name="vO_sb")
        nc.gpsimd.memset(vE_sb[:, :, :, DK:P], 1.0)
        nc.gpsimd.memset(vO_sb[:, :, :, 0:DK], 1.0)
        attnT_sb = persist.tile([P, 2, S], BF16, name="attnT_sb")

        with (
            tc.tile_pool(name="xp", bufs=1) as xp,
            tc.tile_pool(name="e1p", bufs=4) as e1p,
            tc.tile_pool(name="s2p", bufs=2) as s2p,
            tc.tile_pool(name="e2p", bufs=3) as e2p,
            tc.tile_pool(name="drp", bufs=2) as drp,
            tc.tile_pool(name="drbp", bufs=2) as drbp,
            tc.tile_pool(name="d2sp", bufs=2) as d2sp,
            tc.tile_pool(name="otp", bufs=3) as otp,
            tc.tile_pool(name="psSC", bufs=2, space="PSUM") as psSC,
            tc.tile_pool(name="psD1", bufs=2, space="PSUM") as psD1,
            tc.tile_pool(name="psPV", bufs=2, space="PSUM") as psPV,
        ):
            x_sb = xp.tile([P, 8, S], BF16, name="x_sb")
            xTr = xT[:].rearrange("(o p) s -> p o s", p=P)
            for st in range(NB):
                for o in range(8):
                    nc.sync.dma_start(x_sb[:, o, st * WIN:(st + 1) * WIN],
                                      xTr[:, o, st * WIN:(st + 1) * WIN])

            def qk_proj(w_sb, b_sb, dst, st):
                ps = psSC.tile([P, 2, WIN], F32, name="sc_ps")
                for dc in range(2):
                    for o in range(8):
                        nc.tensor.matmul(ps[:, dc, :],
                                         w_sb[:, o, dc * P:(dc + 1) * P],
                                         x_sb[:, o, st * WIN:(st + 1) * WIN],
                                         start=(o == 0), stop=(o == 7))
                    nc.vector.tensor_scalar_add(
                        dst[:, dc, st * WIN:(st + 1) * WIN], ps[:, dc, :],
                        b_sb[:, dc:dc + 1])

            def v_proj(sc):
                ps = psD1.tile([P, WIN], F32, name="d1_ps")
                for o in range(8):
                    nc.tensor.matmul(ps[:, 0:DCORE],
                                     x_sb[:, o, sc * P:(sc + 1) * P],
                                     wv_sb[:, o, :],
                                     start=(o == 0), stop=(o == 7))
                for hc in range(2):
                    e0 = (2 * hc) * DK
                    o0 = (2 * hc + 1) * DK
                    nc.vector.tensor_tensor(vE_sb[:, sc, hc, 0:DK],
                                            ps[:, e0:e0 + DK],
                                            bvr_sb[:, e0:e0 + DK], ADD)
                    nc.vector.tensor_tensor(vO_sb[:, sc, hc, DK:P],
                                            ps[:, o0:o0 + DK],
                                            bvr_sb[:, o0:o0 + DK], ADD)

            def proj_round(r):
                qk_proj(wq_sb, bq_sb, qT_sb, r)
                qk_proj(wk_sb, bk_sb, kT_sb, r)
                for sc in range(4 * r, 4 * r + 4):
                    v_proj(sc)

            state = {}

            def stage_a(job):
                h, bi, j = job
                hc, hb = h // 2, (h % 2) * DK
                diag = (j == bi)
                e1 = e1p.tile([P, NB, WIN], BF16, name="e1")
                for pair in range(2):
                    sc_ps = psSC.tile([P, 2, WIN], F32, name="sc_ps")
                    for mloc in range(2):
                        m = 2 * pair + mloc
                        lo = m * P if diag else 0
                        lhsT = kT_sb[hb:hb + DK, hc,
                                     j * WIN + m * P: j * WIN + (m + 1) * P]
                        rhs = qT_sb[hb:hb + DK, hc,
                                    bi * WIN + lo:(bi + 1) * WIN]
                        nc.tensor.matmul(sc_ps[:, mloc, lo:], lhsT, rhs,
                                         start=True, stop=True)
                    if diag:
                        for mloc in range(2):
                            m = 2 * pair + mloc
                            lo = m * P
                            nc.scalar.activation(e1[:, m, lo:],
                                                 sc_ps[:, mloc, lo:], EXP)
                    else:
                        nc.scalar.activation(e1[:, 2 * pair:2 * pair + 2, :],
                                             sc_ps[:], EXP)
                if diag:
                    for m in range(NB):
                        lo = m * P
                        nc.gpsimd.tensor_tensor(e1[:, m, lo:lo + P],
                                                e1[:, m, lo:lo + P],
                                                mask_sb[:], MULT)
                state[job] = e1

            def stage_b1a(job):
                h, bi, j = job
                diag = (j == bi)
                e1 = state[job]
                d1_ps = psD1.tile([P, WIN], F32, name="d1_ps")
                for m in range(NB):
                    lo = m * P if diag else 0
                    nc.tensor.matmul(d1_ps[:, lo:], ones128[:], e1[:, m, lo:],
                                     start=(m == 0), stop=(m == 3))
                d1r = drp.tile([P, WIN], F32, name="d1r")
                nc.vector.reciprocal_approx_fast(d1r[:], d1_ps[:])
                d1rb = drbp.tile([P, WIN], BF16, name="d1rb")
                nc.gpsimd.tensor_copy(d1rb[:], d1r[:])
                state[(job, "dr")] = d1rb

            def stage_b1b(job):
                h, bi, j = job
                diag = (j == bi)
                e1 = state.pop(job)
                d1rb = state.pop((job, "dr"))
                s2 = s2p.tile([P, NB, WIN], BF16, name="s2")
                e2 = e2p.tile([P, NB, WIN], BF16, name="e2")
                for m in range(NB):
                    lo = m * P if diag else 0
                    nc.vector.tensor_tensor(s2[:, m, lo:], e1[:, m, lo:],
                                            d1rb[:, lo:], MULT)
                if diag:
                    for m in range(1, NB):
                        nc.gpsimd.memset(e2[:, m, 0:m * P], 1.0)
                    for m in range(NB):
                        lo = m * P
                        nc.scalar.activation(e2[:, m, lo:], s2[:, m, lo:], EXP)
                else:
                    nc.scalar.activation(e2[:], s2[:], EXP)
                state[job] = e2

            def stage_b2(job):
                h, bi, j = job
                hc, hb = h // 2, (h % 2) * DK
                vh = vE_sb if h % 2 == 0 else vO_sb
                e2 = state.pop(job)
                if j == 0:
                    state[(h, bi, "pv")] = psPV.tile([P, WIN], F32, name="pv_ps")
                pv_ps = state[(h, bi, "pv")]
                first = (j == 0)
                last = (j == bi)
                for m in range(NB):
                    nc.tensor.matmul(pv_ps[:, :], vh[:, j * 4 + m, hc, :],
                                     e2[:, m, :],
                                     start=(first and m == 0),
                                     stop=(last and m == 3))
                if not last:
                    return
                # fixup: attnT = (pv + sfx) / (d2 + 512*(3-bi))
                pv_ps = state.pop((h, bi, "pv"))
                d2s = d2sp.tile([P, WIN], F32, name="d2s")
                d2r = d2sp.tile([P, WIN], F32, name="d2r")
                cst = float(WIN * (NB - 1 - bi))
                opp = DK - hb  # d2 rows live at the opposite 64-row half
                nc.scalar.activation(d2s[0:DK, :], pv_ps[opp:opp + DK, :],
                                     COPY, bias=cst)
                nc.vector.reciprocal_approx_fast(d2r[0:DK, :], d2s[0:DK, :])
                nc.vector.scalar_tensor_tensor(
                    attnT_sb[hb:hb + DK, hc, bi * WIN:(bi + 1) * WIN],
                    pv_ps[hb:hb + DK, :],
                    sfx_sb[0:DK, hb // DK, hc, bi:bi + 1],
                    d2r[0:DK, :], ADD, MULT)

            def out_proj(st):
                # output projection for query block st; needs attnT[:, :, st]
                for ec in range(8):
                    ps = psD1.tile([P, WIN], F32, name="d1_ps")
                    for dsub in range(2):
                        nc.tensor.matmul(
                            ps[:], wo_sb[:, dsub, ec * P:(ec + 1) * P],
                            attnT_sb[:, dsub, st * WIN:(st + 1) * WIN],
                            start=(dsub == 0), stop=(dsub == 1))
                    ot = otp.tile([P, WIN], BF16, name="ot")
                    nc.vector.tensor_copy(ot[:], ps[:])
                    nc.sync.dma_start(
                        outT[ec * P:(ec + 1) * P, st * WIN:(st + 1) * WIN],
                        ot[:])

            # rounds: proj block r, then jobs (h, r, j<=r), then out block r
            jobs = [(h, bi, j) for bi in range(NB) for h in range(HPC)
                    for j in range(bi + 1)]
            n = len(jobs)
            proj_before = {0: 0, 4: 1, 12: 2, 24: 3}
            outp_after = {3: 0, 11: 1, 23: 2, 39: 3}
            for k in range(n + 3):
                if k in proj_before:
                    proj_round(proj_before[k])
                if k < n:
                    stage_a(jobs[k])
                if 0 <= k - 1 < n:
                    stage_b1a(jobs[k - 1])
                if 0 <= k - 2 < n:
                    stage_b1b(jobs[k - 2])
                if 0 <= k - 3 < n:
                    stage_b2(jobs[k - 3])
                if (k - 3) in outp_after:
                    out_proj(outp_after[k - 3])

    nc.compile()
    return nc


def make_in_maps(x, Wq_w, Wq_b, Wk_w, Wk_b, Wv_w, Wv_b, Wo_w, Wo_b):
    from ml_dtypes import bfloat16

    def bfc(a):
        return np.ascontiguousarray(np.asarray(a, np.float32).astype(bfloat16))

    x = np.asarray(x, np.float32)
    Wq8 = np.asarray(Wq_w, np.float32) / 8.0
    bq8 = np.asarray(Wq_b, np.float32) / 8.0
    wqT = bfc(Wq8.T)
    wkT = bfc(np.asarray(Wk_w, np.float32).T)
    wvT = bfc(np.asarray(Wv_w, np.float32).T)
    woT = bfc(np.asarray(Wo_w, np.float32).T)

    # lower-triangle [128,128] mask for the in-chunk diagonal (k<=q)
    tri = (np.arange(P)[:, None] <= np.arange(P)[None, :]).astype(np.float32)
    maskb = bfc(tri)

    xTb = [bfc(x[b].T) for b in range(B)]

    in_maps = []
    for core in range(NCORES):
        b = core // 4
        h0 = (core % 4) * HPC
        dsl = slice(h0 * DK, (h0 + HPC) * DK)
        bv_core = np.asarray(Wv_b, np.float32)[dsl]
        # suffix colsum(v) table computed on host from the rounded operands:
        # colsum_j(v) = (sum_{s in block j} x[s,:]) @ WvT_core + 512*bv
        wvT_core = np.ascontiguousarray(wvT[:, dsl]).astype(np.float32)
        xb32 = xTb[b].astype(np.float32)
        rowsum = np.stack([xb32[:, j * WIN:(j + 1) * WIN].sum(axis=1)
                           for j in range(NB)])            # [NB, D]
        cs = rowsum @ wvT_core + WIN * bv_core[None, :]     # [NB, DCORE]
        sfx_full = np.zeros((NB, DCORE), np.float32)
        for bi in range(NB - 1):
            sfx_full[bi] = cs[bi + 1:].sum(axis=0)
        sfx = np.zeros((DK, 2, 2, NB), np.float32)
        for hc in range(2):
            for half in range(2):
                for bi in range(NB):
                    sfx[:, half, hc, bi] = sfx_full[bi][
                        hc * P + half * DK: hc * P + half * DK + DK]
        in_maps.append({
            "xT": xTb[b],
            "wqT": np.ascontiguousarray(wqT[:, dsl]),
            "wkT": np.ascontiguousarray(wkT[:, dsl]),
            "wvT": np.ascontiguousarray(wvT[:, dsl]),
            "woT": np.ascontiguousarray(woT[dsl, :]),
            "bq": np.ascontiguousarray(bq8[dsl]),
            "bk": np.ascontiguousarray(np.asarray(Wk_b, np.float32)[dsl]),
            "bvr": np.ascontiguousarray(np.broadcast_to(bv_core, (P, DCORE))),
            "maskd": maskb,
            "sfxd": sfx,
        })
    return in_maps


def kernel(**inputs):
    if "nc" not in _CACHE:
        _CACHE["nc"] = build_nc()
    nc = _CACHE["nc"]
    in_maps = make_in_maps(**inputs)
    kw = {}
    if TRACE:
        kw["trace"] = True
        if TRACE_CORES is not None:
            kw["trace_cores"] = TRACE_CORES
    res = run_bass_kernel_spmd(nc, in_maps, list(range(NCORES)), **kw)
    _CACHE["last_result"] = res

    bo = np.asarray(inputs["Wo_b"], np.float32)
    out = np.zeros((B, S, D), np.float32)
    for b in range(B):
        acc = np.zeros((D, S), np.float32)
        for core in range(b * 4, b * 4 + 4):
            acc += np.asarray(res.results[core]["outT"], np.float32)
        out[b] = acc.T + bo
    return out


# revision 10
# speedup vs baseline: 1.8015x; 1.0363x over previous
"""Trainium2 Bass kernel for nn_MultiHeadAttention_39135742001649.

Reference computation (B=2, S=2048, D=1024, H=16, WIN=512):
    q/k/v = x @ W.T + b (per-head dk=64)
    scores = q k^T / 8                               [B,H,S,S]
    probs1 = blockwise softmax: causal mask, softmax within each 512-wide
             column block (masked entries -> 0)
    probs2 = full-row softmax(probs1)  (no masking; exp(0)=1 entries!)
    out    = (probs2 @ v) @ Wo.T + bo

Decomposition (validated vs reference):
    e1   = exp(scores) * tril_mask        (only 10 of 16 causal blocks)
    d1   = colsum of e1 within block      -> probs1 = e1 / d1
    e2   = exp(probs1)                    (masked/uncomputed entries -> 1)
    out_row = (sum_causal e2 @ v + suffix_colsum_v) / (sum_causal e2 + 512*(3-bi))

Sharding: 8 cores = 2 batches x 4 head-groups (4 heads each). Each core
computes q^T/k^T/v for its heads, the attention, and a partial output
projection over its 256 d-rows; the host sums the 4 partials per batch.

Structure: work proceeds in 4 "rounds", one per query block bi (ascending).
Round r projects q(block r), k(block r), v(chunks 4r..4r+4), then runs all
attention jobs (h, bi=r, j<=r) through a 4-stage skewed pipeline
(scores+exp1 | d1+recip | s2+exp2 | PV+fixup), then the output projection
for block r.  This keeps the in-order PE queue fed from ~20us onward.

Data path is bf16 (PSUM accumulation fp32). 1/d via DVE
reciprocal_approx_fast (no activation-table swaps).
"""

import numpy as np
from contextlib import ExitStack

import concourse.bass as bass
import concourse.mybir as mybir
import concourse.tile as tile
from concourse import bacc
from concourse.bass_utils import run_bass_kernel_spmd

F32 = mybir.dt.float32
BF16 = mybir.dt.bfloat16
EXP = mybir.ActivationFunctionType.Exp
COPY = mybir.ActivationFunctionType.Copy
IDENT = mybir.ActivationFunctionType.Identity
ADD = mybir.AluOpType.add
MULT = mybir.AluOpType.mult
BYPASS = mybir.AluOpType.bypass

B, S, D, H, WIN = 2, 2048, 1024, 16, 512
DK = D // H          # 64
NB = S // WIN        # 4
NCORES = 8
HPC = 4              # heads per core
DCORE = HPC * DK     # 256
P = 128

TRACE = False        # set True from test.py to capture HW profile
TRACE_CORES = None

_CACHE = {}


def build_nc():
    nc = bacc.Bacc("TRN2", target_bir_lowering=False, debug=False)

    xT = nc.dram_tensor("xT", [D, S], BF16, kind="ExternalInput")        # x[b].T
    wqT = nc.dram_tensor("wqT", [D, DCORE], BF16, kind="ExternalInput")  # (Wq/8).T slice
    wkT = nc.dram_tensor("wkT", [D, DCORE], BF16, kind="ExternalInput")
    wvT = nc.dram_tensor("wvT", [D, DCORE], BF16, kind="ExternalInput")
    woT = nc.dram_tensor("woT", [DCORE, D], BF16, kind="ExternalInput")  # Wo.T row slice
    bq = nc.dram_tensor("bq", [DCORE], F32, kind="ExternalInput")        # /8
    bk = nc.dram_tensor("bk", [DCORE], F32, kind="ExternalInput")
    bvr = nc.dram_tensor("bvr", [P, DCORE], F32, kind="ExternalInput")   # bv replicated
    maskd = nc.dram_tensor("maskd", [P, P], BF16, kind="ExternalInput")  # tril triangle
    sfxd = nc.dram_tensor("sfxd", [DK, 2, 2, NB], F32, kind="ExternalInput")
    outT = nc.dram_tensor("outT", [D, S], BF16, kind="ExternalOutput")   # partial out^T

    with tile.TileContext(nc) as tc, ExitStack() as ctx:
        const = ctx.enter_context(tc.tile_pool(name="const", bufs=1))
        wpool = ctx.enter_context(tc.tile_pool(name="wpool", bufs=1))
        persist = ctx.enter_context(tc.tile_pool(name="persist", bufs=1))

        mask_sb = const.tile([P, P], BF16, name="mask_sb")
        nc.sync.dma_start(mask_sb[:], maskd[:])
        bq_sb = const.tile([P, 2], F32, name="bq_sb")
        nc.sync.dma_start(bq_sb[:], bq[:].rearrange("(c p) -> p c", p=P))
        bk_sb = const.tile([P, 2], F32, name="bk_sb")
        nc.sync.dma_start(bk_sb[:], bk[:].rearrange("(c p) -> p c", p=P))
        bvr_sb = const.tile([P, DCORE], F32, name="bvr_sb")
        nc.sync.dma_start(bvr_sb[:], bvr[:])
        sfx_sb = const.tile([DK, 2, 2, NB], F32, name="sfx_sb")  # suffix sums
        nc.sync.dma_start(sfx_sb[:], sfxd[:])

        ones128 = const.tile([P, P], BF16, name="ones128")
        nc.gpsimd.memset(ones128[:], 1.0)

        wq_sb = wpool.tile([P, 8, DCORE], BF16, name="wq_sb")
        wk_sb = wpool.tile([P, 8, DCORE], BF16, name="wk_sb")
        wv_sb = wpool.tile([P, 8, DCORE], BF16, name="wv_sb")
        wo_sb = wpool.tile([P, 2, D], BF16, name="wo_sb")
        nc.sync.dma_start(wq_sb[:], wqT[:].rearrange("(o p) d -> p o d", p=P))

        qT_sb = persist.tile([P, 2, S], BF16, name="qT_sb")    # [d%128, d//128, s]
        kT_sb = persist.tile([P, 2, S], BF16, name="kT_sb")
        # Per head-pair padded V tiles for the PV matmul: even head's v in
        # cols 0:64 with ones in 64:128 (d2 lands in psum rows 64:128);
        # odd head's v in cols 64:128 with ones in 0:64 (d2 in rows 0:64).
        vE_sb = persist.tile([P, 16, 2, P], BF16, name="vE_sb")
        vO_sb = persist.tile([P, 16, 2, P], BF16, name="vO_sb")
        nc.gpsimd.memset(vE_sb[:, :, :, DK:P], 1.0)
        nc.gpsimd.memset(vO_sb[:, :, :, 0:DK], 1.0)
        attnT_sb = persist.tile([P, 2, S], BF16, name="attnT_sb")

        with (
            tc.tile_pool(name="xp", bufs=1) as xp,
            tc.tile_pool(name="e1p", bufs=5) as e1p,
            tc.tile_pool(name="s2p", bufs=2) as s2p,
            tc.tile_pool(name="e2p", bufs=4) as e2p,
            tc.tile_pool(name="drp", bufs=2) as drp,
            tc.tile_pool(name="drbp", bufs=3) as drbp,
            tc.tile_pool(name="d2sp", bufs=2) as d2sp,
            tc.tile_pool(name="otp", bufs=3) as otp,
            tc.tile_pool(name="psSC", bufs=2, space="PSUM") as psSC,
            tc.tile_pool(name="psD1", bufs=2, space="PSUM") as psD1,
            tc.tile_pool(name="psPV", bufs=2, space="PSUM") as psPV,
        ):
            x_sb = xp.tile([P, 8, S], BF16, name="x_sb")
            xTr = xT[:].rearrange("(o p) s -> p o s", p=P)
            for st in range(NB):
                for o in range(8):
                    nc.sync.dma_start(x_sb[:, o, st * WIN:(st + 1) * WIN],
                                      xTr[:, o, st * WIN:(st + 1) * WIN])
                if st == 0:
                    nc.sync.dma_start(
                        wk_sb[:], wkT[:].rearrange("(o p) d -> p o d", p=P))
                    nc.sync.dma_start(
                        wv_sb[:], wvT[:].rearrange("(o p) d -> p o d", p=P))
                elif st == 1:
                    nc.sync.dma_start(
                        wo_sb[:], woT[:].rearrange("(o p) e -> p o e", p=P))

            def qk_proj(w_sb, b_sb, dst, st):
                ps = psSC.tile([P, 2, WIN], F32, name="sc_ps")
                for dc in range(2):
                    for o in range(8):
                        nc.tensor.matmul(ps[:, dc, :],
                                         w_sb[:, o, dc * P:(dc + 1) * P],
                                         x_sb[:, o, st * WIN:(st + 1) * WIN],
                                         start=(o == 0), stop=(o == 7))
                    nc.vector.tensor_scalar_add(
                        dst[:, dc, st * WIN:(st + 1) * WIN], ps[:, dc, :],
                        b_sb[:, dc:dc + 1])

            def v_proj(sc):
                ps = psD1.tile([P, WIN], F32, name="d1_ps")
                for o in range(8):
                    nc.tensor.matmul(ps[:, 0:DCORE],
                                     x_sb[:, o, sc * P:(sc + 1) * P],
                                     wv_sb[:, o, :],
                                     start=(o == 0), stop=(o == 7))
                for hc in range(2):
                    e0 = (2 * hc) * DK
                    o0 = (2 * hc + 1) * DK
                    nc.vector.tensor_tensor(vE_sb[:, sc, hc, 0:DK],
                                            ps[:, e0:e0 + DK],
                                            bvr_sb[:, e0:e0 + DK], ADD)
                    nc.vector.tensor_tensor(vO_sb[:, sc, hc, DK:P],
                                            ps[:, o0:o0 + DK],
                                            bvr_sb[:, o0:o0 + DK], ADD)

            def proj_round(r):
                qk_proj(wq_sb, bq_sb, qT_sb, r)
                qk_proj(wk_sb, bk_sb, kT_sb, r)
                for sc in range(4 * r, 4 * r + 4):
                    v_proj(sc)

            state = {}

            def stage_a(job):
                h, bi, j = job
                hc, hb = h // 2, (h % 2) * DK
                diag = (j == bi)
                e1 = e1p.tile([P, NB, WIN], BF16, name="e1")
                for pair in range(2):
                    sc_ps = psSC.tile([P, 2, WIN], F32, name="sc_ps")
                    for mloc in range(2):
                        m = 2 * pair + mloc
                        lo = m * P if diag else 0
                        lhsT = kT_sb[hb:hb + DK, hc,
                                     j * WIN + m * P: j * WIN + (m + 1) * P]
                        rhs = qT_sb[hb:hb + DK, hc,
                                    bi * WIN + lo:(bi + 1) * WIN]
                        nc.tensor.matmul(sc_ps[:, mloc, lo:], lhsT, rhs,
                                         start=True, stop=True)
                    if diag:
                        for mloc in range(2):
                            m = 2 * pair + mloc
                            lo = m * P
                            nc.scalar.activation(e1[:, m, lo:],
                                                 sc_ps[:, mloc, lo:], EXP)
                    else:
                        nc.scalar.activation(e1[:, 2 * pair:2 * pair + 2, :],
                                             sc_ps[:], EXP)
                if diag:
                    for m in range(NB):
                        lo = m * P
                        nc.gpsimd.tensor_tensor(e1[:, m, lo:lo + P],
                                                e1[:, m, lo:lo + P],
                                                mask_sb[:], MULT)
                state[job] = e1

            def stage_b1a(job):
                h, bi, j = job
                diag = (j == bi)
                e1 = state[job]
                d1_ps = psD1.tile([P, WIN], F32, name="d1_ps")
                for m in range(NB):
                    lo = m * P if diag else 0
                    nc.tensor.matmul(d1_ps[:, lo:], ones128[:], e1[:, m, lo:],
                                     start=(m == 0), stop=(m == 3))
                d1r = drp.tile([P, WIN], F32, name="d1r")
                nc.vector.reciprocal_approx_fast(d1r[:], d1_ps[:])
                d1rb = drbp.tile([P, WIN], BF16, name="d1rb")
                nc.gpsimd.tensor_copy(d1rb[:], d1r[:])
                state[(job, "dr")] = d1rb

            def stage_b1b(job):
                h, bi, j = job
                diag = (j == bi)
                e1 = state.pop(job)
                d1rb = state.pop((job, "dr"))
                s2 = s2p.tile([P, NB, WIN], BF16, name="s2")
                e2 = e2p.tile([P, NB, WIN], BF16, name="e2")
                for m in range(NB):
                    lo = m * P if diag else 0
                    nc.vector.tensor_tensor(s2[:, m, lo:], e1[:, m, lo:],
                                            d1rb[:, lo:], MULT)
                if diag:
                    for m in range(1, NB):
                        nc.gpsimd.memset(e2[:, m, 0:m * P], 1.0)
                    for m in range(NB):
                        lo = m * P
                        nc.scalar.activation(e2[:, m, lo:], s2[:, m, lo:], EXP)
                else:
                    nc.scalar.activation(e2[:], s2[:], EXP)
                state[job] = e2

            def stage_b2(job):
                h, bi, j = job
                hc, hb = h // 2, (h % 2) * DK
                vh = vE_sb if h % 2 == 0 else vO_sb
                e2 = state.pop(job)
                if j == 0:
                    state[(h, bi, "pv")] = psPV.tile([P, WIN], F32, name="pv_ps")
                pv_ps = state[(h, bi, "pv")]
                first = (j == 0)
                last = (j == bi)
                for m in range(NB):
                    nc.tensor.matmul(pv_ps[:, :], vh[:, j * 4 + m, hc, :],
                                     e2[:, m, :],
                                     start=(first and m == 0),
                                     stop=(last and m == 3))
                if not last:
                    return
                # fixup: attnT = (pv + sfx) / (d2 + 512*(3-bi))
                pv_ps = state.pop((h, bi, "pv"))
                d2s = d2sp.tile([P, WIN], F32, name="d2s")
                d2r = d2sp.tile([P, WIN], F32, name="d2r")
                cst = float(WIN * (NB - 1 - bi))
                opp = DK - hb  # d2 rows live at the opposite 64-row half
                nc.scalar.activation(d2s[0:DK, :], pv_ps[opp:opp + DK, :],
                                     COPY, bias=cst)
                nc.vector.reciprocal_approx_fast(d2r[0:DK, :], d2s[0:DK, :])
                nc.vector.scalar_tensor_tensor(
                    attnT_sb[hb:hb + DK, hc, bi * WIN:(bi + 1) * WIN],
                    pv_ps[hb:hb + DK, :],
                    sfx_sb[0:DK, hb // DK, hc, bi:bi + 1],
                    d2r[0:DK, :], ADD, MULT)

            def out_proj(st):
                # output projection for query block st; needs attnT[:, :, st]
                for ec in range(8):
                    ps = psD1.tile([P, WIN], F32, name="d1_ps")
                    for dsub in range(2):
                        nc.tensor.matmul(
                            ps[:], wo_sb[:, dsub, ec * P:(ec + 1) * P],
                            attnT_sb[:, dsub, st * WIN:(st + 1) * WIN],
                            start=(dsub == 0), stop=(dsub == 1))
                    ot = otp.tile([P, WIN], BF16, name="ot")
                    nc.vector.tensor_copy(ot[:], ps[:])
                    nc.sync.dma_start(
                        outT[ec * P:(ec + 1) * P, st * WIN:(st + 1) * WIN],
                        ot[:])

            # rounds: proj block r, then jobs (h, r, j<=r), then out block r
            jobs = [(h, bi, j) for bi in range(NB) for h in range(HPC)
                    for j in range(bi + 1)]
            n = len(jobs)
            proj_before = {0: 0, 4: 1, 12: 2, 24: 3}
            outp_after = {3: 0, 11: 1, 23: 2, 39: 3}
            for k in range(n + 5):
                if k in proj_before:
                    proj_round(proj_before[k])
                if k < n:
                    stage_a(jobs[k])
                if 0 <= k - 2 < n:
                    stage_b1a(jobs[k - 2])
                if 0 <= k - 3 < n:
                    stage_b1b(jobs[k - 3])
                if 0 <= k - 5 < n:
                    stage_b2(jobs[k - 5])
                if (k - 5) in outp_after:
                    out_proj(outp_after[k - 5])

    nc.compile()
    return nc


def make_in_maps(x, Wq_w, Wq_b, Wk_w, Wk_b, Wv_w, Wv_b, Wo_w, Wo_b):
    from ml_dtypes import bfloat16

    def bfc(a):
        return np.ascontiguousarray(np.asarray(a, np.float32).astype(bfloat16))

    x = np.asarray(x, np.float32)
    Wq8 = np.asarray(Wq_w, np.float32) / 8.0
    bq8 = np.asarray(Wq_b, np.float32) / 8.0
    wqT = bfc(Wq8.T)
    wkT = bfc(np.asarray(Wk_w, np.float32).T)
    wvT = bfc(np.asarray(Wv_w, np.float32).T)
    woT = bfc(np.asarray(Wo_w, np.float32).T)

    # lower-triangle [128,128] mask for the in-chunk diagonal (k<=q)
    tri = (np.arange(P)[:, None] <= np.arange(P)[None, :]).astype(np.float32)
    maskb = bfc(tri)

    xTb = [bfc(x[b].T) for b in range(B)]

    in_maps = []
    for core in range(NCORES):
        b = core // 4
        h0 = (core % 4) * HPC
        dsl = slice(h0 * DK, (h0 + HPC) * DK)
        bv_core = np.asarray(Wv_b, np.float32)[dsl]
        # suffix colsum(v) table computed on host from the rounded operands:
        # colsum_j(v) = (sum_{s in block j} x[s,:]) @ WvT_core + 512*bv
        wvT_core = np.ascontiguousarray(wvT[:, dsl]).astype(np.float32)
        xb32 = xTb[b].astype(np.float32)
        rowsum = np.stack([xb32[:, j * WIN:(j + 1) * WIN].sum(axis=1)
                           for j in range(NB)])            # [NB, D]
        cs = rowsum @ wvT_core + WIN * bv_core[None, :]     # [NB, DCORE]
        sfx_full = np.zeros((NB, DCORE), np.float32)
        for bi in range(NB - 1):
            sfx_full[bi] = cs[bi + 1:].sum(axis=0)
        sfx = np.zeros((DK, 2, 2, NB), np.float32)
        for hc in range(2):
            for half in range(2):
                for bi in range(NB):
                    sfx[:, half, hc, bi] = sfx_full[bi][
                        hc * P + half * DK: hc * P + half * DK + DK]
        in_maps.append({
            "xT": xTb[b],
            "wqT": np.ascontiguousarray(wqT[:, dsl]),
            "wkT": np.ascontiguousarray(wkT[:, dsl]),
            "wvT": np.ascontiguousarray(wvT[:, dsl]),
            "woT": np.ascontiguousarray(woT[dsl, :]),
            "bq": np.ascontiguousarray(bq8[dsl]),
            "bk": np.ascontiguousarray(np.asarray(Wk_b, np.float32)[dsl]),
            "bvr": np.ascontiguousarray(np.broadcast_to(bv_core, (P, DCORE))),
            "maskd": maskb,
            "sfxd": sfx,
        })
    return in_maps


def kernel(**inputs):
    if "nc" not in _CACHE:
        _CACHE["nc"] = build_nc()
    nc = _CACHE["nc"]
    in_maps = make_in_maps(**inputs)
    kw = {}
    if TRACE:
        kw["trace"] = True
        if TRACE_CORES is not None:
            kw["trace_cores"] = TRACE_CORES
    res = run_bass_kernel_spmd(nc, in_maps, list(range(NCORES)), **kw)
    _CACHE["last_result"] = res

    bo = np.asarray(inputs["Wo_b"], np.float32)
    out = np.zeros((B, S, D), np.float32)
    for b in range(B):
        acc = np.zeros((D, S), np.float32)
        for core in range(b * 4, b * 4 + 4):
            acc += np.asarray(res.results[core]["outT"], np.float32)
        out[b] = acc.T + bo
    return out
